# revision 24
# baseline (speedup 1.0000x reference)
"""ContextualLoss forward on 8 trn2 NeuronCores — single-matmul-pass version.

Problem: X, Y [4, 256, 64, 64] f32 ->  loss [4] f32
  y_mean[c] = mean_hw(Y);  Xc = X - y_mean; Yc = Y - y_mean
  Xn, Yn: L2-normalized over C per spatial position; S = Xn^T @ Yn  [N, N]
  d = 1 - S; dmin = row min d; w = exp((1 - d/(dmin+1e-3))/0.1); A = w/rowsum(w)
  loss_b = -log(mean_n max_m A[n, m])

Algebra (per row n, g = 1/||Xc_n||, S = Xc^T @ Yn with Yn = Yc*invnY):
  max_m A[n,:] = 1 / Z'[n],   Z'[n] = sum_m exp((S[n,m] - smax[n]) * s[n])
  smax = row max S,  ndm = 1.001 - smax*g  (= dmin + 1e-3),  s = 10*g/ndm.
(The softmax ratio is invariant to common additive shifts in the exponent,
so the exact d/dmin form reduces to this shifted-scaled one.)

Engine plan per core (4 samples x 2 row-halves across 8 cores):
  PE    single bf16 matmul pass -> PSUM [128,2048] halves (2 bufs = 8 banks),
        matmuls grouped by stationary operand (2 LDWEIGHTS per half).
  DVE   row max from PSUM (FD=2048) + copies cols [0,CD) of each half to the
        SBUF f32 slab + the per-block reciprocal.
  ACT   copies cols [CD,2048) of each half PSUM->SBUF (Identity), per-block
        ndm, then one Exp over the [128,4096] slab with per-row scale/bias
        and accum_out = Z'.  All funcs in one table set (natural_log_exp).
  Pool  invnY broadcast + the three per-block [128,1] multiplies.
Host combines: loss_b = -log((sum of cores' [128,1] outputs)/4096).
"""

import numpy as np

B, C, HW = 4, 256, 4096
HALF = HW // 2
NCORES = 8
NB = HALF // 128      # 16 row blocks per core
H_INV = 10.0          # 1/h with h = 0.1
CD = 600              # columns per half copied by DVE (rest by ACT)

_nc_cache = None


def _build():
    import concourse.bass as bass
    import concourse.bacc as bacc
    import concourse.tile as tile
    from concourse import mybir

    f32 = mybir.dt.float32
    bf16 = mybir.dt.bfloat16
    AF = mybir.ActivationFunctionType
    OP = mybir.AluOpType
    AX = mybir.AxisListType

    nc = bacc.Bacc(None)

    y_dram = nc.dram_tensor("y", [C, HW], f32, kind="ExternalInput")
    x_dram = nc.dram_tensor("xh", [C, HALF], f32, kind="ExternalInput")
    out_dram = nc.dram_tensor("out", [128, 1], f32, kind="ExternalOutput")

    with tile.TileContext(nc) as tc:
        with (
            tc.tile_pool(name="persist", bufs=1) as P,
            tc.tile_pool(name="stats", bufs=3) as ST,
        ):
            # ---------------- constants / persistent tiles ----------------
            ones_mm = P.tile([128, 1], bf16)
            nc.vector.memset(ones_mm, 1.0)
            negones = P.tile([128, 1], f32)
            nc.vector.memset(negones, -1.0)
            c1001 = P.tile([128, 1], f32)
            nc.vector.memset(c1001, 1.001)
            # pin the ACT table set: Ln first narrows the chooser to the
            # natural_log_exp set which also holds Square/Identity/Exp.
            tbl = P.tile([128, 1], f32)
            nc.scalar.activation(out=tbl, in_=c1001, func=AF.Ln, bias=0.0, scale=1.0)

            yn = [P.tile([128, HW], bf16, tag=f"yn{i}", name=f"yn{i}") for i in range(2)]
            xcb = [P.tile([128, HALF], bf16, tag=f"xcb{i}", name=f"xcb{i}") for i in range(2)]
            g10 = P.tile([128, NB], f32, tag="g10")      # 10 * invnX
            gneg = P.tile([128, NB], f32, tag="gneg")    # -invnX
            zallD = P.tile([128, NB], f32, tag="zallD")
            zallA = P.tile([128, NB], f32, tag="zallA")
            negmean = [P.tile([128, 1], f32, tag=f"nm{i}", name=f"nm{i}") for i in range(2)]

            # ---------------- setup (freed before main loop) ----------------
            with (
                tc.tile_pool(name="setup", bufs=1) as SU,
                tc.tile_pool(name="sups", bufs=1, space="PSUM") as SUPS,
            ):
                y_sb = [SU.tile([128, HW], f32, tag=f"y{i}", name=f"y{i}") for i in range(2)]
                x_sb = [SU.tile([128, HALF], f32, tag=f"x{i}", name=f"x{i}") for i in range(2)]
                for i in range(2):
                    for ch in range(4):
                        sl = slice(ch * 1024, (ch + 1) * 1024)
                        nc.sync.dma_start(out=y_sb[i][:, sl], in_=y_dram[128 * i : 128 * (i + 1), sl])
                for i in range(2):
                    nc.sync.dma_start(out=x_sb[i], in_=x_dram[128 * i : 128 * (i + 1), :])

                # per-channel spatial mean of Y: tile0 on DVE (chunked
                # behind DMA), tile1 via ACT Identity+accum in parallel
                ysp = SU.tile([128, 4], f32, tag="ysp")
                for ch in range(4):
                    sl = slice(ch * 1024, (ch + 1) * 1024)
                    nc.vector.reduce_sum(out=ysp[:, ch : ch + 1], in_=y_sb[0][:, sl], axis=AX.X)
                ys_0 = SU.tile([128, 1], f32, tag="ys0")
                nc.vector.reduce_sum(out=ys_0, in_=ysp, axis=AX.X)
                nc.vector.tensor_scalar_mul(out=negmean[0], in0=ys_0, scalar1=-1.0 / HW)
                ytrash = SU.tile([128, HW], bf16, tag="ytrash")
                ys_1 = SU.tile([128, 1], f32, tag="ys1")
                nc.scalar.activation(out=ytrash, in_=y_sb[1], func=AF.Identity, bias=0.0, scale=1.0, accum_out=ys_1)
                nc.vector.tensor_scalar_mul(out=negmean[1], in0=ys_1, scalar1=-1.0 / HW)

                # centered X in bf16 + its squares, all on DVE (ACT is the
                # critical resource in the normalization chain below)
                for i in range(2):
                    nc.vector.scalar_tensor_tensor(
                        out=xcb[i], in0=x_sb[i], scalar=negmean[i], in1=x_sb[i],
                        op0=OP.add, op1=OP.bypass,
                    )
                xsq = [SU.tile([128, HALF], bf16, tag=f"xsq{i}", name=f"xsq{i}") for i in range(2)]
                for i in range(2):
                    nc.vector.tensor_tensor(out=xsq[i], in0=xcb[i], in1=xcb[i], op=OP.mult)

                # Y normalization chain, pipelined per 1024-column chunk:
                # Square (ACT) -> ones-matmul (PE) -> Ln (ACT); then the X
                # norm Ln; then one table switch and all the Exps; the
                # partition broadcast (Pool) and yn production (DVE) chase
                # the Exp chunks so the main loop can start early.
                ysq = [SU.tile([128, HW], bf16, tag=f"ysq{i}", name=f"ysq{i}") for i in range(2)]
                lny = SU.tile([1, HW], f32, tag="lny")
                for c in range(4):
                    sl = slice(c * 1024, (c + 1) * 1024)
                    for i in range(2):
                        nc.scalar.activation(out=ysq[i][:, sl], in_=y_sb[i][:, sl], func=AF.Square, bias=negmean[i], scale=1.0)
                    ssy_c = SUPS.tile([1, 1024], f32, tag="ssyc", name=f"ssyc{c}")
                    for cc in range(2):
                        psl = slice(cc * 512, (cc + 1) * 512)
                        gsl = slice(c * 1024 + cc * 512, c * 1024 + (cc + 1) * 512)
                        nc.tensor.matmul(ssy_c[:, psl], ones_mm, ysq[0][:, gsl], start=True, stop=False)
                        nc.tensor.matmul(ssy_c[:, psl], ones_mm, ysq[1][:, gsl], start=False, stop=True)
                    nc.scalar.activation(out=lny[:, sl], in_=ssy_c, func=AF.Ln, bias=0.0, scale=1.0)

                ssxT = SUPS.tile([128, 16], f32, tag="ssx")
                for pb in range(16):
                    psl = slice(pb * 128, (pb + 1) * 128)
                    nc.tensor.matmul(ssxT[:, pb : pb + 1], xsq[0][:, psl], ones_mm, start=True, stop=False)
                    nc.tensor.matmul(ssxT[:, pb : pb + 1], xsq[1][:, psl], ones_mm, start=False, stop=True)
                lnx = SU.tile([128, 16], f32, tag="lnx")
                nc.scalar.activation(out=lnx, in_=ssxT, func=AF.Ln, bias=0.0, scale=1.0)

                invny_row = SU.tile([1, HW], f32, tag="invnyr")
                invny_b = SU.tile([128, HW], f32, tag="invnyb")
                for c in range(4):
                    sl = slice(c * 1024, (c + 1) * 1024)
                    nc.scalar.activation(out=invny_row[:, sl], in_=lny[:, sl], func=AF.Exp, bias=0.0, scale=-0.5)
                    for bb in range(2):
                        bsl = slice(c * 1024 + bb * 512, c * 1024 + (bb + 1) * 512)
                        nc.gpsimd.partition_broadcast(invny_b[:, bsl], invny_row[0:1, bsl])
                    for i in range(2):
                        nc.vector.scalar_tensor_tensor(
                            out=yn[i][:, sl], in0=y_sb[i][:, sl], scalar=negmean[i],
                            in1=invny_b[:, sl], op0=OP.add, op1=OP.mult,
                        )
                invnxT = SU.tile([128, 16], f32, tag="invnxT")
                nc.scalar.activation(out=invnxT, in_=lnx, func=AF.Exp, bias=0.0, scale=-0.5)
                nc.vector.tensor_scalar_mul(out=g10, in0=invnxT, scalar1=H_INV)
                nc.vector.tensor_scalar_mul(out=gneg, in0=invnxT, scalar1=-1.0)

            # ---------------- main loop over 16 row blocks ----------------
            # Pure two-pass with quarter tiles, pass B delayed one block:
            #   pass A (psA, DVE-only): row max of the 4 quarters
            #   pass B (psB, ACT-only): recompute + Exp straight from PSUM
            # PE is the ~95%-busy pacer (stays HAM-warm); the stats chain
            # (combine -> ndm -> 1/ndm -> Pool products) has a full block of
            # slack before its exps consume it.  A and B matmuls are
            # interleaved so psB recycling (gated by the previous block's
            # exps) never stalls PE.
            QW = 1024  # quarter width
            with (
                tc.tile_pool(name="psA", bufs=2, space="PSUM") as PSA,
                tc.tile_pool(name="psB", bufs=2, space="PSUM") as PSB,
            ):
                zall4 = P.tile([128, 4 * NB], f32, tag="zall4")

                def emit_A_mms(nb, q, pa):
                    nsl = slice(nb * 128, (nb + 1) * 128)
                    for ci in range(2):
                        for cc in range(2):
                            csl = slice(cc * 512, (cc + 1) * 512)
                            msl = slice(q * QW + cc * 512, q * QW + (cc + 1) * 512)
                            nc.tensor.matmul(
                                pa[:, csl], xcb[ci][:, nsl], yn[ci][:, msl],
                                start=(ci == 0), stop=(ci == 1),
                            )

                def emit_B_q(nb, q, scol, bcol, slab=None):
                    nsl = slice(nb * 128, (nb + 1) * 128)
                    if slab is None:
                        pb = PSB.tile([128, QW], f32, tag="pb", name=f"pb{nb}_{q}")
                        for ci in range(2):
                            for cc in range(2):
                                csl = slice(cc * 512, (cc + 1) * 512)
                                msl = slice(q * QW + cc * 512, q * QW + (cc + 1) * 512)
                                nc.tensor.matmul(
                                    pb[:, csl], xcb[ci][:, nsl], yn[ci][:, msl],
                                    start=(ci == 0), stop=(ci == 1),
                                )
                        src_ap = pb
                    else:
                        src_ap = slab
                    dj = DU_pool.tile([128, QW], bf16, tag="d0", name=f"d{nb}_{q}")
                    nc.scalar.activation(
                        out=dj, in_=src_ap, func=AF.Exp, bias=bcol, scale=scol,
                        accum_out=zall4[:, 4 * nb + q : 4 * nb + q + 1],
                    )

                with (
                    tc.tile_pool(name="dumps", bufs=3) as DU_pool,
                    tc.tile_pool(name="slab", bufs=3) as SL_pool,
                ):
                    pend = None
                    for nb in range(NB):
                        mx4 = ST.tile([128, 4], f32, tag="mx4")
                        smax = ST.tile([128, 1], f32, tag="smax")
                        slabD = SL_pool.tile([128, QW], f32, tag="sd")
                        for q in range(4):
                            pa = PSA.tile([128, QW], f32, tag="pa", name=f"pa{nb}_{q}")
                            emit_A_mms(nb, q, pa)
                            if pend is not None:
                                emit_B_q(pend[2], q, pend[0], pend[1], slab=(pend[3] if q == 0 else None))
                            nc.vector.reduce_max(out=mx4[:, q : q + 1], in_=pa, axis=AX.X)
                            if q == 0:
                                nc.vector.tensor_copy(slabD, pa)
                        nc.vector.reduce_max(out=smax, in_=mx4, axis=AX.X)
                        ndm = ST.tile([128, 1], f32, tag="ndm")
                        nc.vector.scalar_tensor_tensor(
                            out=ndm, in0=smax, scalar=gneg[:, nb : nb + 1], in1=c1001,
                            op0=OP.mult, op1=OP.add,
                        )
                        rr = ST.tile([128, 1], f32, tag="rr")
                        nc.vector.reciprocal(out=rr, in_=ndm)
                        scol = ST.tile([128, 1], f32, tag="sc")
                        nc.gpsimd.tensor_tensor(out=scol, in0=rr, in1=g10[:, nb : nb + 1], op=OP.mult)
                        t0 = ST.tile([128, 1], f32, tag="t0")
                        nc.gpsimd.tensor_tensor(out=t0, in0=smax, in1=scol, op=OP.mult)
                        bcol = ST.tile([128, 1], f32, tag="bc")
                        nc.gpsimd.tensor_tensor(out=bcol, in0=t0, in1=negones, op=OP.mult)
                        pend = (scol, bcol, nb, slabD)
                    for q in range(4):
                        emit_B_q(pend[2], q, pend[0], pend[1], slab=(pend[3] if q == 0 else None))

                    # ---------------- epilogue: sum_n 1/Z ----------------
                    zall = P.tile([128, NB], f32, tag="zall")
                    nc.vector.reduce_sum(
                        out=zall, in_=zall4.rearrange("p (nb j) -> p nb j", j=4), axis=AX.X
                    )
                    rz = P.tile([128, NB], f32, tag="rz")
                    nc.vector.reciprocal(out=rz, in_=zall)
                    acc = P.tile([128, 1], f32, tag="acc")
                    nc.vector.reduce_sum(out=acc, in_=rz, axis=AX.X)
                    nc.gpsimd.dma_start(out=out_dram[:, :], in_=acc)

    nc.finalize()
    return nc


def _get_nc():
    global _nc_cache
    if _nc_cache is None:
        _nc_cache = _build()
    return _nc_cache


def run_cores(inputs, **kwargs):
    """Run the 8-core SPMD kernel; returns (loss[4], BassKernelResults)."""
    from concourse.bass_utils import run_bass_kernel_spmd

    nc = _get_nc()
    X = np.asarray(inputs["X_features"], dtype=np.float32).reshape(B, C, HW)
    Y = np.asarray(inputs["Y_features"], dtype=np.float32).reshape(B, C, HW)
    in_maps = []
    for core in range(NCORES):
        b, h = divmod(core, 2)
        in_maps.append(
            {
                "y": np.ascontiguousarray(Y[b]),
                "xh": np.ascontiguousarray(X[b, :, h * HALF : (h + 1) * HALF]),
            }
        )
    res = run_bass_kernel_spmd(nc, in_maps, core_ids=list(range(NCORES)), **kwargs)
    acc = np.stack(
        [res.results[i]["out"].reshape(-1).astype(np.float64) for i in range(NCORES)]
    )  # [8, 128]
    cx = acc.reshape(B, 2 * 128).sum(axis=1) / HW
    loss = (-np.log(cx)).astype(np.float32)
    return loss, res


def kernel(**inputs):
    return run_cores(inputs)[0]


# revision 27
# speedup vs baseline: 1.0323x; 1.0323x over previous
"""ContextualLoss forward on 8 trn2 NeuronCores — single-matmul-pass version.

Problem: X, Y [4, 256, 64, 64] f32 ->  loss [4] f32
  y_mean[c] = mean_hw(Y);  Xc = X - y_mean; Yc = Y - y_mean
  Xn, Yn: L2-normalized over C per spatial position; S = Xn^T @ Yn  [N, N]
  d = 1 - S; dmin = row min d; w = exp((1 - d/(dmin+1e-3))/0.1); A = w/rowsum(w)
  loss_b = -log(mean_n max_m A[n, m])

Algebra (per row n, g = 1/||Xc_n||, S = Xc^T @ Yn with Yn = Yc*invnY):
  max_m A[n,:] = 1 / Z'[n],   Z'[n] = sum_m exp((S[n,m] - smax[n]) * s[n])
  smax = row max S,  ndm = 1.001 - smax*g  (= dmin + 1e-3),  s = 10*g/ndm.
(The softmax ratio is invariant to common additive shifts in the exponent,
so the exact d/dmin form reduces to this shifted-scaled one.)

Engine plan per core (4 samples x 2 row-halves across 8 cores):
  PE    single bf16 matmul pass -> PSUM [128,2048] halves (2 bufs = 8 banks),
        matmuls grouped by stationary operand (2 LDWEIGHTS per half).
  DVE   row max from PSUM (FD=2048) + copies cols [0,CD) of each half to the
        SBUF f32 slab + the per-block reciprocal.
  ACT   copies cols [CD,2048) of each half PSUM->SBUF (Identity), per-block
        ndm, then one Exp over the [128,4096] slab with per-row scale/bias
        and accum_out = Z'.  All funcs in one table set (natural_log_exp).
  Pool  invnY broadcast + the three per-block [128,1] multiplies.
Host combines: loss_b = -log((sum of cores' [128,1] outputs)/4096).
"""

import numpy as np

B, C, HW = 4, 256, 4096
HALF = HW // 2
NCORES = 8
NB = HALF // 128      # 16 row blocks per core
H_INV = 10.0          # 1/h with h = 0.1
CD = 600              # columns per half copied by DVE (rest by ACT)

_nc_cache = None


def _build():
    import concourse.bass as bass
    import concourse.bacc as bacc
    import concourse.tile as tile
    from concourse import mybir

    f32 = mybir.dt.float32
    bf16 = mybir.dt.bfloat16
    AF = mybir.ActivationFunctionType
    OP = mybir.AluOpType
    AX = mybir.AxisListType

    nc = bacc.Bacc(None)

    y_dram = nc.dram_tensor("y", [C, HW], f32, kind="ExternalInput")
    x_dram = nc.dram_tensor("xh", [C, HALF], f32, kind="ExternalInput")
    out_dram = nc.dram_tensor("out", [128, 1], f32, kind="ExternalOutput")

    with tile.TileContext(nc) as tc:
        with (
            tc.tile_pool(name="persist", bufs=1) as P,
            tc.tile_pool(name="stats", bufs=3) as ST,
        ):
            # ---------------- constants / persistent tiles ----------------
            ones_mm = P.tile([128, 1], bf16)
            nc.vector.memset(ones_mm, 1.0)
            negones = P.tile([128, 1], f32)
            nc.vector.memset(negones, -1.0)
            c1001 = P.tile([128, 1], f32)
            nc.vector.memset(c1001, 1.001)
            # pin the ACT table set: Ln first narrows the chooser to the
            # natural_log_exp set which also holds Square/Identity/Exp.
            tbl = P.tile([128, 1], f32)
            nc.scalar.activation(out=tbl, in_=c1001, func=AF.Ln, bias=0.0, scale=1.0)

            yn = [P.tile([128, HW], bf16, tag=f"yn{i}", name=f"yn{i}") for i in range(2)]
            xcb = [P.tile([128, HALF], bf16, tag=f"xcb{i}", name=f"xcb{i}") for i in range(2)]
            g10 = P.tile([128, NB], f32, tag="g10")      # 10 * invnX
            gneg = P.tile([128, NB], f32, tag="gneg")    # -invnX
            zallD = P.tile([128, NB], f32, tag="zallD")
            zallA = P.tile([128, NB], f32, tag="zallA")
            negmean = [P.tile([128, 1], f32, tag=f"nm{i}", name=f"nm{i}") for i in range(2)]

            # ---------------- setup (freed before main loop) ----------------
            with (
                tc.tile_pool(name="setup", bufs=1) as SU,
                tc.tile_pool(name="sups", bufs=1, space="PSUM") as SUPS,
            ):
                y_sb = [SU.tile([128, HW], f32, tag=f"y{i}", name=f"y{i}") for i in range(2)]
                x_sb = [SU.tile([128, HALF], f32, tag=f"x{i}", name=f"x{i}") for i in range(2)]
                for i in range(2):
                    for ch in range(4):
                        sl = slice(ch * 1024, (ch + 1) * 1024)
                        nc.sync.dma_start(out=y_sb[i][:, sl], in_=y_dram[128 * i : 128 * (i + 1), sl])
                for i in range(2):
                    nc.sync.dma_start(out=x_sb[i], in_=x_dram[128 * i : 128 * (i + 1), :])

                # per-channel spatial mean of Y: tile0 on DVE (chunked
                # behind DMA), tile1 via ACT Identity+accum in parallel
                ysp = SU.tile([128, 4], f32, tag="ysp")
                for ch in range(4):
                    sl = slice(ch * 1024, (ch + 1) * 1024)
                    nc.vector.reduce_sum(out=ysp[:, ch : ch + 1], in_=y_sb[0][:, sl], axis=AX.X)
                ys_0 = SU.tile([128, 1], f32, tag="ys0")
                nc.vector.reduce_sum(out=ys_0, in_=ysp, axis=AX.X)
                nc.vector.tensor_scalar_mul(out=negmean[0], in0=ys_0, scalar1=-1.0 / HW)
                ytrash = SU.tile([128, HW], bf16, tag="ytrash")
                ys_1 = SU.tile([128, 1], f32, tag="ys1")
                nc.scalar.activation(out=ytrash, in_=y_sb[1], func=AF.Identity, bias=0.0, scale=1.0, accum_out=ys_1)
                nc.vector.tensor_scalar_mul(out=negmean[1], in0=ys_1, scalar1=-1.0 / HW)

                # centered X in bf16 + its squares, all on DVE (ACT is the
                # critical resource in the normalization chain below)
                for i in range(2):
                    nc.vector.scalar_tensor_tensor(
                        out=xcb[i], in0=x_sb[i], scalar=negmean[i], in1=x_sb[i],
                        op0=OP.add, op1=OP.bypass,
                    )
                xsq = [SU.tile([128, HALF], bf16, tag=f"xsq{i}", name=f"xsq{i}") for i in range(2)]
                for i in range(2):
                    nc.vector.tensor_tensor(out=xsq[i], in0=xcb[i], in1=xcb[i], op=OP.mult)

                # Y normalization chain, pipelined per 1024-column chunk:
                # Square (ACT) -> ones-matmul (PE) -> Ln (ACT); then the X
                # norm Ln; then one table switch and all the Exps; the
                # partition broadcast (Pool) and yn production (DVE) chase
                # the Exp chunks so the main loop can start early.
                ysq = [SU.tile([128, HW], bf16, tag=f"ysq{i}", name=f"ysq{i}") for i in range(2)]
                lny = SU.tile([1, HW], f32, tag="lny")
                for c in range(4):
                    sl = slice(c * 1024, (c + 1) * 1024)
                    for i in range(2):
                        nc.scalar.activation(out=ysq[i][:, sl], in_=y_sb[i][:, sl], func=AF.Square, bias=negmean[i], scale=1.0)
                    ssy_c = SUPS.tile([1, 1024], f32, tag="ssyc", name=f"ssyc{c}")
                    for cc in range(2):
                        psl = slice(cc * 512, (cc + 1) * 512)
                        gsl = slice(c * 1024 + cc * 512, c * 1024 + (cc + 1) * 512)
                        nc.tensor.matmul(ssy_c[:, psl], ones_mm, ysq[0][:, gsl], start=True, stop=False)
                        nc.tensor.matmul(ssy_c[:, psl], ones_mm, ysq[1][:, gsl], start=False, stop=True)
                    nc.scalar.activation(out=lny[:, sl], in_=ssy_c, func=AF.Ln, bias=0.0, scale=1.0)

                ssxT = SUPS.tile([128, 16], f32, tag="ssx")
                for pb in range(16):
                    psl = slice(pb * 128, (pb + 1) * 128)
                    nc.tensor.matmul(ssxT[:, pb : pb + 1], xsq[0][:, psl], ones_mm, start=True, stop=False)
                    nc.tensor.matmul(ssxT[:, pb : pb + 1], xsq[1][:, psl], ones_mm, start=False, stop=True)
                lnx = SU.tile([128, 16], f32, tag="lnx")
                nc.scalar.activation(out=lnx, in_=ssxT, func=AF.Ln, bias=0.0, scale=1.0)

                invny_row = SU.tile([1, HW], f32, tag="invnyr")
                invny_b = SU.tile([128, HW], f32, tag="invnyb")
                for c in range(4):
                    sl = slice(c * 1024, (c + 1) * 1024)
                    nc.scalar.activation(out=invny_row[:, sl], in_=lny[:, sl], func=AF.Exp, bias=0.0, scale=-0.5)
                    for bb in range(2):
                        bsl = slice(c * 1024 + bb * 512, c * 1024 + (bb + 1) * 512)
                        nc.gpsimd.partition_broadcast(invny_b[:, bsl], invny_row[0:1, bsl])
                    for i in range(2):
                        nc.vector.scalar_tensor_tensor(
                            out=yn[i][:, sl], in0=y_sb[i][:, sl], scalar=negmean[i],
                            in1=invny_b[:, sl], op0=OP.add, op1=OP.mult,
                        )
                invnxT = SU.tile([128, 16], f32, tag="invnxT")
                nc.scalar.activation(out=invnxT, in_=lnx, func=AF.Exp, bias=0.0, scale=-0.5)
                nc.vector.tensor_scalar_mul(out=g10, in0=invnxT, scalar1=H_INV)
                nc.vector.tensor_scalar_mul(out=gneg, in0=invnxT, scalar1=-1.0)

            # ---------------- main loop over 16 row blocks ----------------
            # Pure two-pass with quarter tiles, pass B delayed one block:
            #   pass A (psA, DVE-only): row max of the 4 quarters
            #   pass B (psB, ACT-only): recompute + Exp straight from PSUM
            # PE is the ~95%-busy pacer (stays HAM-warm); the stats chain
            # (combine -> ndm -> 1/ndm -> Pool products) has a full block of
            # slack before its exps consume it.  A and B matmuls are
            # interleaved so psB recycling (gated by the previous block's
            # exps) never stalls PE.
            QW = 1024  # quarter width
            with (
                tc.tile_pool(name="psA", bufs=2, space="PSUM") as PSA,
                tc.tile_pool(name="psB", bufs=2, space="PSUM") as PSB,
            ):
                zall4 = P.tile([128, 4 * NB], f32, tag="zall4")

                def emit_A_mms(nb, q, pa):
                    nsl = slice(nb * 128, (nb + 1) * 128)
                    for ci in range(2):
                        for cc in range(2):
                            csl = slice(cc * 512, (cc + 1) * 512)
                            msl = slice(q * QW + cc * 512, q * QW + (cc + 1) * 512)
                            nc.tensor.matmul(
                                pa[:, csl], xcb[ci][:, nsl], yn[ci][:, msl],
                                start=(ci == 0), stop=(ci == 1),
                            )

                def emit_B_q(nb, q, scol, bcol):
                    nsl = slice(nb * 128, (nb + 1) * 128)
                    pb = PSB.tile([128, QW], f32, tag="pb", name=f"pb{nb}_{q}")
                    for ci in range(2):
                        for cc in range(2):
                            csl = slice(cc * 512, (cc + 1) * 512)
                            msl = slice(q * QW + cc * 512, q * QW + (cc + 1) * 512)
                            nc.tensor.matmul(
                                pb[:, csl], xcb[ci][:, nsl], yn[ci][:, msl],
                                start=(ci == 0), stop=(ci == 1),
                            )
                    dj = DU_pool.tile([128, QW], bf16, tag="d0", name=f"d{nb}_{q}")
                    nc.scalar.activation(
                        out=dj, in_=pb, func=AF.Exp, bias=bcol, scale=scol,
                        accum_out=zall4[:, 4 * nb + q : 4 * nb + q + 1],
                    )

                with tc.tile_pool(name="dumps", bufs=3) as DU_pool:
                    pend = None
                    for nb in range(NB):
                        mx4 = ST.tile([128, 4], f32, tag="mx4")
                        smax = ST.tile([128, 1], f32, tag="smax")
                        for q in range(4):
                            pa = PSA.tile([128, QW], f32, tag="pa", name=f"pa{nb}_{q}")
                            emit_A_mms(nb, q, pa)
                            if pend is not None:
                                emit_B_q(pend[2], q, pend[0], pend[1])
                            nc.vector.reduce_max(out=mx4[:, q : q + 1], in_=pa, axis=AX.X)
                        nc.vector.reduce_max(out=smax, in_=mx4, axis=AX.X)
                        ndm = ST.tile([128, 1], f32, tag="ndm")
                        nc.vector.scalar_tensor_tensor(
                            out=ndm, in0=smax, scalar=gneg[:, nb : nb + 1], in1=c1001,
                            op0=OP.mult, op1=OP.add,
                        )
                        rr = ST.tile([128, 1], f32, tag="rr")
                        nc.vector.reciprocal(out=rr, in_=ndm)
                        scol = ST.tile([128, 1], f32, tag="sc")
                        nc.gpsimd.tensor_tensor(out=scol, in0=rr, in1=g10[:, nb : nb + 1], op=OP.mult)
                        t0 = ST.tile([128, 1], f32, tag="t0")
                        nc.gpsimd.tensor_tensor(out=t0, in0=smax, in1=scol, op=OP.mult)
                        bcol = ST.tile([128, 1], f32, tag="bc")
                        nc.gpsimd.tensor_tensor(out=bcol, in0=t0, in1=negones, op=OP.mult)
                        pend = (scol, bcol, nb)
                    for q in range(4):
                        emit_B_q(pend[2], q, pend[0], pend[1])

                    # ---------------- epilogue: sum_n 1/Z ----------------
                    zall = P.tile([128, NB], f32, tag="zall")
                    nc.vector.reduce_sum(
                        out=zall, in_=zall4.rearrange("p (nb j) -> p nb j", j=4), axis=AX.X
                    )
                    rz = P.tile([128, NB], f32, tag="rz")
                    nc.vector.reciprocal(out=rz, in_=zall)
                    acc = P.tile([128, 1], f32, tag="acc")
                    nc.vector.reduce_sum(out=acc, in_=rz, axis=AX.X)
                    nc.gpsimd.dma_start(out=out_dram[:, :], in_=acc)

    nc.finalize()
    return nc


def _get_nc():
    global _nc_cache
    if _nc_cache is None:
        _nc_cache = _build()
    return _nc_cache


def run_cores(inputs, **kwargs):
    """Run the 8-core SPMD kernel; returns (loss[4], BassKernelResults)."""
    from concourse.bass_utils import run_bass_kernel_spmd

    nc = _get_nc()
    X = np.asarray(inputs["X_features"], dtype=np.float32).reshape(B, C, HW)
    Y = np.asarray(inputs["Y_features"], dtype=np.float32).reshape(B, C, HW)
    in_maps = []
    for core in range(NCORES):
        b, h = divmod(core, 2)
        in_maps.append(
            {
                "y": np.ascontiguousarray(Y[b]),
                "xh": np.ascontiguousarray(X[b, :, h * HALF : (h + 1) * HALF]),
            }
        )
    res = run_bass_kernel_spmd(nc, in_maps, core_ids=list(range(NCORES)), **kwargs)
    acc = np.stack(
        [res.results[i]["out"].reshape(-1).astype(np.float64) for i in range(NCORES)]
    )  # [8, 128]
    cx = acc.reshape(B, 2 * 128).sum(axis=1) / HW
    loss = (-np.log(cx)).astype(np.float32)
    return loss, res


def kernel(**inputs):
    return run_cores(inputs)[0]
